# revision 24
# baseline (speedup 1.0000x reference)
"""Trainium2 Bass kernel for nn_CantorGlobalAttention (v3: grid interp).

Math (per dir d, expert e, batch b):
    logits[p, k] = Q[d,e,b,p] * S[d,e,b,k],   k = (w, p') in [0, 768)
    attn = softmax_k(logits);  att[p, :] = attn[p, :] @ Vn[k, :]
    out[b, e*P+p, :] = sum_d softmax(fusion_w)[d] * att[d, ...]

Key structure: logits are rank-1, so the attended row for query p is a
smooth function of the SCALAR t = q_p:

    g(t) = F(t) / Z(t),  F(t) = sum_k e^{t s_k} Vn_k,  Z(t) = sum_k e^{t s_k}

Each component of g is a ratio of sums of pure exponentials e^{t s_k} with
|s| <= ~6.3 here, so on a uniform t-grid with step h a 6-tap (quintic)
Lagrange interpolation is accurate to ~0.005*(h*|s|max)^6 relative — below
the bf16-V noise floor for G=64 grid points covering [min q, max q] per
(d,e,b).  So instead of P=256 queries we evaluate the attention at G=64
grid points (4x fewer exps — exp on ACT at 1 elem/lane/cycle is the hard
bottleneck of the direct method) and reconstruct all 256 rows with a small
dense fp16 interp matmul whose quintic weights are built on the host
(data-dependent VALUES, static SHAPES -> SPMD-safe).

The grid exponent is further factored e^{t_i s} = e^{(i-GMID) h s} *
e^{tmid s}: the device computes only the iota part (a single [128, G] iota
tile broadcast via 0-stride APs; no per-group broadcast-q DMA at all), and
the host folds e^{tmid s} into vp's V rows and its Z column.  This keeps
the DMA stream at ~20MB/core — the kernel is DMA-stream-bound, so bytes
are the speed currency.

Per group g=(i expert, d dir), all wide single instructions:
  DVE   L[k,(c,b,i)] = iota[i] * hs[k,(c,b)] : one [128, 3072] fp16
        tensor_tensor with 0-stride broadcast APs (hs materialized x4 on
        the host so the last AP dim stays stride-1 and the DVE 2x fp16
        mode applies).
  ACT   EG = exp(L): one wide [128, 3072] activation -> bf16.
  PE    per batch pair: two 6-chunk chains into one 2-bank PSUM tile
        (start=True zeroing is bank-scoped; verified on HW):
        FZ[i_grid, 0:128 | 128] = EG^T @ [fold*V | fold].
  norm  at GRID level, per pair: one reciprocal [G,2] + one tensor_tensor
        with a 0-stride-broadcast reciprocal -> fp16 Fg (g(t_i) is O(1),
        so fp16 is safe — this is why the grid is normalized before
        interpolation).
  interp once per expert: per (b, j): 5-matmul fp16 chain accumulating the
        DIRECTION SUM directly in PSUM: out_j += Wt[d,b,j]^T @ Fg[d,b]
        with fusion weights folded into Wt on the host.  Final [128,128]
        PSUM->SBUF copies (split ACT/DVE) -> fp16 output DMA.  Interp
        chains are dripped between grid chains (pending queue) so phase-2
        work never bunches at expert boundaries.

No per-query softmax normalization anywhere, no collectives.

Sharding: expert-parallel, 2 experts per core (core c owns experts 2c,
2c+1). Outputs land in disjoint slots of the [B, E*P, D] output.
"""

import os
import sys

import numpy as np

sys.path.insert(0, "/opt/trn_rl_repo")

import concourse.bass as bass  # noqa: E402
import concourse.tile as tile  # noqa: E402
from concourse import bacc  # noqa: E402
from concourse import mybir  # noqa: E402
from concourse import bass_utils  # noqa: E402

try:
    from ml_dtypes import bfloat16 as _bf16
except ImportError:  # pragma: no cover
    _bf16 = None

# Problem shape (fixed by the nn.Module).
N_DIR, E, B, P, D, W = 5, 16, 8, 256, 128, 3
EPS = 1e-6
N_CORES = 8
EPC = E // N_CORES          # experts per core = 2
NG = EPC * N_DIR            # groups per core = 10, group g = (i, d)
K = W * P                   # 768 routed keys per query
NCH = K // 128              # 6 k-chunks of 128 partitions
FB = B * P                  # 2048
NT = NCH * B                # 48 V tiles per group
VW = 129                    # V tile width: 128 dcols + ones column
G = 64                      # t-grid points per (d, e, b)
GMID = (G - 1) / 2.0        # iota center
GF = NCH * B * G            # 4608 = logit/exp free size per group
REP = 4                     # host-side s replication factor (96 = 24*4)

F32 = mybir.dt.float32
BF16 = mybir.dt.bfloat16
F16 = mybir.dt.float16

# Exposed for test.py: set True to collect an NTFF profile.
PROFILE = False
LAST_EXEC_NS = None
LAST_TRACE = None

_PROGRAM_CACHE = {}

_AXON_SO = "/opt/axon/libaxon_pjrt.so"


def _ensure_ntff_hook():
    """The container image ships a slim ``antenv`` without ``axon_hooks``;
    register an equivalent module backed by ctypes calls into
    libaxon_pjrt.so so run_bass_kernel_spmd(trace=True) can profile."""
    import sys as _sys
    if "antenv.axon_hooks" in _sys.modules:
        return
    import contextlib
    import ctypes
    import types

    try:
        lib = ctypes.CDLL(_AXON_SO)
    except OSError:
        return
    if not hasattr(lib, "axon_start_nrt_profile"):
        return
    lib.axon_start_nrt_profile.argtypes = [
        ctypes.POINTER(ctypes.c_int64), ctypes.c_size_t]
    lib.axon_start_nrt_profile.restype = ctypes.c_int64
    lib.axon_stop_nrt_profile.argtypes = [ctypes.c_char_p]
    lib.axon_stop_nrt_profile.restype = ctypes.c_int64

    @contextlib.contextmanager
    def _hook(output_dir, device_ids):
        import jax
        jax.devices()
        if device_ids:
            ids = (ctypes.c_int64 * len(device_ids))(*device_ids)
            rc = lib.axon_start_nrt_profile(ids, len(device_ids))
        else:
            rc = lib.axon_start_nrt_profile(None, 0)
        if rc != 0:
            raise RuntimeError(f"axon_start_nrt_profile rc={rc}")
        try:
            yield
        finally:
            n = lib.axon_stop_nrt_profile(str(output_dir).encode())
            print(f"ntff profile: {n} file(s) -> {output_dir}")

    mod = types.ModuleType("antenv.axon_hooks")
    mod.get_axon_ntff_profile_hook = lambda: _hook
    mod.set_axon_ntff_profile_hook = lambda h: None
    _sys.modules["antenv.axon_hooks"] = mod


def build_program(bias_c):
    """Build the SPMD Bass/Tile program (identical on all 8 cores)."""
    from contextlib import ExitStack

    nc = bacc.Bacc("TRN2", target_bir_lowering=False, debug=False,
                   num_devices=N_CORES)

    iot_d = nc.dram_tensor("iot", [128, G], F16, kind="ExternalInput")
    sr_d = nc.dram_tensor("sr", [128, NG * NCH * B * REP], F16,
                          kind="ExternalInput")
    vp_d = nc.dram_tensor("vp", [NG, 128, NT * VW], BF16, kind="ExternalInput")
    wt_d = nc.dram_tensor("wt", [G, NG * FB], F16, kind="ExternalInput")
    out_d = nc.dram_tensor("out", [B, EPC * P, D], F16, kind="ExternalOutput")

    LAG = 2                 # stage B (PE/norm) trails stage A by 2 groups

    with tile.TileContext(nc) as tc, ExitStack() as ctx:
        iot_pool = ctx.enter_context(tc.tile_pool(name="iot", bufs=1))
        sr_pool = ctx.enter_context(tc.tile_pool(name="sr", bufs=1))
        wt_pool = ctx.enter_context(tc.tile_pool(name="wt", bufs=2))
        v_pool = ctx.enter_context(tc.tile_pool(name="vp", bufs=LAG + 3))
        l_pool = ctx.enter_context(tc.tile_pool(name="logit", bufs=2))
        e_pool = ctx.enter_context(tc.tile_pool(name="expt", bufs=LAG + 3))
        fg_pool = ctx.enter_context(tc.tile_pool(name="fg", bufs=40))
        rz_pool = ctx.enter_context(tc.tile_pool(name="rz", bufs=12))
        fo_pool = ctx.enter_context(tc.tile_pool(name="fout", bufs=6))
        gps_pool = ctx.enter_context(
            tc.tile_pool(name="gpsum", bufs=3, space="PSUM"))
        ips_pool = ctx.enter_context(
            tc.tile_pool(name="ipsum", bufs=2, space="PSUM"))

        # iota + ALL the (tiny) hs data land up front so the logit/exp
        # pipeline never queues behind the bulk vp stream.
        iot_sb = iot_pool.tile([128, G], F16)
        nc.sync.dma_start(iot_sb[:, :], iot_d[:, :])
        sr_sb = sr_pool.tile([128, NG * NCH * B * REP], F16)
        nc.sync.dma_start(sr_sb[:, :], sr_d[:, :])

        e_tiles = [None] * NG
        v_tiles = [None] * NG
        wt_tiles = [None] * EPC
        fg_tiles = {}            # (g, b) -> [G, 128] fp16 normalized grid
        pending = []             # (expert, b) interps awaiting emission

        PAIRW = 2 * NCH * VW    # one batch-pair's width in the b-major vp

        def emit_dma(g):
            """Prefetch group g's bulk V tiles (ahead of use).  The last
            two groups' vp is DMA'd per batch PAIR so the drain-critical
            grid chains can start on a pair as soon as its slice lands."""
            v_t = v_pool.tile([128, NT * VW], BF16)
            if g >= NG - 2:
                for q in range(4):
                    nc.sync.dma_start(
                        v_t[:, q * PAIRW:(q + 1) * PAIRW],
                        vp_d[g, :, q * PAIRW:(q + 1) * PAIRW])
            else:
                nc.sync.dma_start(v_t[:, :], vp_d[g, :, :])
            v_tiles[g] = v_t
            if g % N_DIR == 4:
                # this expert's interp weights; first used by phase 2 —
                # streamed AFTER the expert's last vp so the compute-critical
                # V bytes are never queued behind it.
                i = g // N_DIR
                wt_t = wt_pool.tile([G, N_DIR * FB], F16)
                nc.sync.dma_start(
                    wt_t[:, :],
                    wt_d[:, i * N_DIR * FB:(i + 1) * N_DIR * FB])
                wt_tiles[i] = wt_t

        def emit_interp(i, b):
            """Quintic interp + direction sum in one PSUM chain, then
            PSUM->SBUF copy and the output DMA, for (expert i, batch b)."""
            wt_sb = wt_tiles[i]
            for j in range(2):
                ps2 = ips_pool.tile([128, 128], F32)
                for d in range(N_DIR):
                    gg = i * N_DIR + d
                    wt_ap = wt_sb[:, (d * B + b) * P + j * 128:
                                  (d * B + b) * P + j * 128 + 128]
                    nc.tensor.matmul(
                        ps2[:, :],
                        wt_ap,
                        fg_tiles[(gg, b)][:, :],
                        start=(d == 0), stop=(d == N_DIR - 1),
                    )
                fo = fo_pool.tile([128, 128], F16)
                if j == 0:
                    nc.scalar.activation(
                        fo[:, :], ps2[:, :],
                        mybir.ActivationFunctionType.Copy,
                        bias=0.0, scale=1.0)
                else:
                    nc.vector.tensor_scalar(
                        fo[:, :], ps2[:, :], 1.0, None,
                        mybir.AluOpType.mult)
                nc.sync.dma_start(
                    out_d[b, i * P + j * 128:i * P + j * 128 + 128, :],
                    fo[:, :])

        emit_dma(0)
        for g in range(NG + LAG):
            if g + 1 < NG:
                emit_dma(g + 1)
            if g < NG:
                # ---- stage A: logits + exp for group g ----
                l_t = l_pool.tile([128, GF], F16)
                # l[k,(c,b,i)] = (i - GMID) * (h_b * s_kcb); the remaining
                # e^{tmid*s} factor of e^{t_i s} is folded into vp's rows
                # (and its Z column) on the host.
                iot_ap = iot_sb[:, :].unsqueeze(1).broadcast_to(
                    [128, NCH * B, G])
                base = g * NCH * B * REP
                sr_ap = sr_sb[:, base:base + NCH * B * REP]
                sr_ap = sr_ap.rearrange("p (cb r) -> p cb r", r=REP)
                sr_ap = sr_ap.unsqueeze(2).broadcast_to(
                    [128, NCH * B, G // REP, REP])
                nc.vector.tensor_tensor(
                    l_t[:, :].rearrange("p (cb i) -> p cb i", i=G),
                    iot_ap, sr_ap, mybir.AluOpType.mult)

                e_t = e_pool.tile([128, GF], BF16)
                nc.scalar.activation(
                    e_t[:, :], l_t[:, :],
                    mybir.ActivationFunctionType.Exp,
                    bias=float(bias_c), scale=1.0,
                )
                e_tiles[g] = e_t

            if g >= LAG:
                # ---- stage B: grid chains + grid-normalize for g-LAG ----
                # Two chains share one 2-bank PSUM tile (chain u=0 in bank 0,
                # u=1 in bank 1; start=True zeroing is bank-scoped), so the
                # recip and the normalize-copy each cover a PAIR of batches.
                gp = g - LAG
                e_t = e_tiles[gp]
                v_t = v_tiles[gp]
                i, d = gp // N_DIR, gp % N_DIR
                for bp in range(0, B, 2):
                    ps = gps_pool.tile([128, 1024], F32)
                    for u in range(2):
                        b = bp + u
                        for c in range(NCH):
                            nc.tensor.matmul(
                                ps[0:G, u * 512:u * 512 + VW],
                                e_t[:, (c * B + b) * G:(c * B + b + 1) * G],
                                v_t[:, (b * NCH + c) * VW:
                                    (b * NCH + c + 1) * VW],
                                start=(c == 0), stop=(c == NCH - 1),
                            )
                    psv = ps[0:G, :].rearrange("p (u v) -> p u v", v=512)
                    rz2 = rz_pool.tile([128, 2], F32)
                    nc.vector.reciprocal(rz2[0:G, :], psv[:, :, 128:129])
                    fg2 = fg_pool.tile([G, 2 * 128], F16)
                    # normalized grid rows: g(t_i) = F/Z, O(1) -> fp16 safe.
                    rz_ap = rz2[0:G, :].unsqueeze(2).broadcast_to(
                        [G, 2, 128])
                    nc.vector.tensor_tensor(
                        fg2[:, :].rearrange("p (u v) -> p u v", v=128),
                        psv[:, :, 0:128], rz_ap, mybir.AluOpType.mult)
                    fg_tiles[(gp, bp)] = fg2[:, 0:128]
                    fg_tiles[(gp, bp + 1)] = fg2[:, 128:256]
                    # Drip pending interp chains between grid chains so the
                    # phase-2 work never bunches at expert boundaries.
                    if gp == NG - 1:
                        pending.append((i, bp))
                        pending.append((i, bp + 1))
                        if bp >= 2:
                            emit_interp(*pending.pop(0))
                            emit_interp(*pending.pop(0))
                    elif pending:
                        emit_interp(*pending.pop(0))
                if d == N_DIR - 1 and gp < NG - 1:
                    pending.extend((i, b) for b in range(B))
                if gp == NG - 1:
                    while pending:
                        emit_interp(*pending.pop(0))

    nc.compile()
    return nc


def host_prep(Q_aff, K_aff, V, betas, temperature, fusion_w, routes):
    """Shard + gather + layout inputs for the 8 cores. Returns
    (in_maps, bias_c)."""
    Q_aff = np.asarray(Q_aff, np.float32)
    K_aff = np.asarray(K_aff, np.float32)
    V = np.asarray(V, np.float32)
    betas = np.asarray(betas, np.float32)
    temperature = np.asarray(temperature, np.float32)
    fusion_w = np.asarray(fusion_w, np.float32)
    routes = np.asarray(routes)

    T = abs(float(temperature[0])) + EPS
    fw = np.exp(fusion_w - fusion_w.max())
    fw = (fw / fw.sum()).astype(np.float32)          # softmax(fusion_w)

    ar = np.arange(E)
    is_self = routes == ar[:, None]
    gates = 1.0 / (1.0 + np.exp(-betas[ar[:, None], routes]))
    beta = np.where(is_self, 1.0, gates).astype(np.float32)   # [E, W]

    # S[d, e, b, k] with k = w*P + p'
    nbK = K_aff[:, routes]                            # [d, E, W, b, P]
    S = nbK * beta[None, :, :, None, None] / np.float32(T)
    S = np.moveaxis(S, 2, 3).reshape(N_DIR, E, B, K)  # [d, E, b, K]

    # t-grids per (d, e, b): G points spanning [qmin, qmax] with 2.5-tap
    # margin so every q_p sits in the interior of a 6-tap stencil.
    qmin = Q_aff.min(axis=3)                          # [d, E, B]
    qmax = Q_aff.max(axis=3)
    h = np.maximum((qmax - qmin) / (G - 6), 1e-5)
    tgrid = (qmin[..., None] + (np.arange(G, dtype=np.float32) - 2.5)
             * h[..., None]).astype(np.float32)       # [d, E, B, G]
    # Factorization e^{t_i s} = e^{(i-47.5) h s} * e^{tmid s} with
    # tmid = t0 + 45h: the device computes only the iota part; the
    # e^{tmid s} factor is folded into vp's V rows and its Z column.
    tmid = (qmin + (GMID - 2.5) * h).astype(np.float32)       # [d, E, B]

    # Max |grid logit| of the iota part: decide the exp shift (bf16 range
    # guard; e^88 overflows bf16).
    sabs = np.abs(S).max(axis=3)
    maxarg = float((GMID * h * sabs).max())
    bias_c = 0.0 if maxarg < 80.0 else -(maxarg - 60.0)

    # Quintic Lagrange interp weights W[p, G] per (d, e, b), scaled by the
    # fusion weight so the direction sum happens inside PSUM chains.
    cell = ((Q_aff - tgrid[..., 0:1]) / h[..., None]).astype(np.int64)
    cell = np.clip(cell, 2, G - 4)                    # [d, E, B, P]
    i0 = cell - 2
    taps = i0[..., None] + np.arange(6)               # [d, E, B, P, 6]
    xs = np.take_along_axis(
        tgrid[..., None, :], taps, axis=4)            # [d, E, B, P, 6]
    q = Q_aff[..., None]                              # [d, E, B, P, 1]
    wq = np.ones((N_DIR, E, B, P, 6), np.float64)
    for a in range(6):
        for c in range(6):
            if c == a:
                continue
            wq[..., a] *= (q[..., 0] - xs[..., c]) / (xs[..., a] - xs[..., c])
    Wfull = np.zeros((N_DIR, E, B, P, G), np.float32)
    np.put_along_axis(Wfull, taps, wq.astype(np.float32), axis=4)
    Wfull *= fw[:, None, None, None, None]

    if _bf16 is None:
        raise RuntimeError("ml_dtypes.bfloat16 required")

    iot = np.broadcast_to(
        (np.arange(G, dtype=np.float32) - GMID).astype(np.float16),
        (128, G)).copy()

    in_maps = []
    for core in range(N_CORES):
        experts = [EPC * core + i for i in range(EPC)]

        sr = np.empty((128, NG * NCH * B * REP), np.float16)
        vp = np.empty((NG, 128, NT, VW), np.float32)
        wt = np.empty((G, NG * FB), np.float16)
        for i, e in enumerate(experts):
            for d in range(N_DIR):
                g = i * N_DIR + d
                # sr holds h_b * s so the iota multiply lands at (i-47.5)*h*s
                hs_mat = (S[d, e] * h[d, e][:, None]).reshape(
                    B, NCH, 128).transpose(2, 1, 0)
                sr[:, g * NCH * B * REP:(g + 1) * NCH * B * REP] = np.repeat(
                    hs_mat.reshape(128, NCH * B).astype(np.float16),
                    REP, axis=1)
                # wt[i_grid, g*FB + b*P + p] = fw[d] * W[d,e,b,p,i_grid]
                wt[:, g * FB:(g + 1) * FB] = (
                    Wfull[d, e].reshape(FB, G).T.astype(np.float16))
                # vp is b-major: tile index = b*NCH + c
                for c in range(NCH):
                    w, half = c // 2, c % 2
                    f = int(routes[e, w])
                    # fold e^{tmid s} into the V rows and the Z column
                    fold = np.exp(
                        tmid[d, e][:, None]
                        * S[d, e, :, c * 128:(c + 1) * 128]
                    ).astype(np.float32)              # [B, 128]
                    vp[g, :, c::NCH, :D] = (
                        fold[:, :, None]
                        * V[d, f, :, half * 128:(half + 1) * 128, :]
                    ).transpose(1, 0, 2)
                    vp[g, :, c::NCH, D] = fold.T
        in_maps.append({
            "iot": iot,
            "sr": sr,
            "vp": vp.reshape(NG, 128, NT * VW).astype(_bf16),
            "wt": wt,
        })
    return in_maps, bias_c


def kernel(**inputs):
    global LAST_EXEC_NS, LAST_TRACE
    in_maps, bias_c = host_prep(**inputs)

    key = (bias_c,)
    nc = _PROGRAM_CACHE.get(key)
    if nc is None:
        nc = build_program(bias_c)
        _PROGRAM_CACHE[key] = nc

    if PROFILE:
        _ensure_ntff_hook()
    res = bass_utils.run_bass_kernel_spmd(
        nc, in_maps, list(range(N_CORES)), trace=PROFILE)
    LAST_EXEC_NS = res.exec_time_ns
    LAST_TRACE = getattr(res, "instructions_and_trace", None)

    out = np.empty((B, E * P, D), np.float32)
    for core in range(N_CORES):
        out[:, EPC * core * P:(EPC * core + EPC) * P, :] = (
            res.results[core]["out"].astype(np.float32))
    return out


# revision 26
# speedup vs baseline: 1.0865x; 1.0865x over previous
"""Trainium2 Bass kernel for nn_CantorGlobalAttention (v3: grid interp).

Math (per dir d, expert e, batch b):
    logits[p, k] = Q[d,e,b,p] * S[d,e,b,k],   k = (w, p') in [0, 768)
    attn = softmax_k(logits);  att[p, :] = attn[p, :] @ Vn[k, :]
    out[b, e*P+p, :] = sum_d softmax(fusion_w)[d] * att[d, ...]

Key structure: logits are rank-1, so the attended row for query p is a
smooth function of the SCALAR t = q_p:

    g(t) = F(t) / Z(t),  F(t) = sum_k e^{t s_k} Vn_k,  Z(t) = sum_k e^{t s_k}

Each component of g is a ratio of sums of pure exponentials e^{t s_k} with
|s| <= ~6.3 here, so on a uniform t-grid with step h a 6-tap (quintic)
Lagrange interpolation is accurate to ~0.005*(h*|s|max)^6 relative — below
the bf16-V noise floor for G=64 grid points covering [min q, max q] per
(d,e,b).  So instead of P=256 queries we evaluate the attention at G=64
grid points (4x fewer exps — exp on ACT at 1 elem/lane/cycle is the hard
bottleneck of the direct method) and reconstruct all 256 rows with a small
dense fp16 interp matmul whose quintic weights are built on the host
(data-dependent VALUES, static SHAPES -> SPMD-safe).

The grid exponent is further factored e^{t_i s} = e^{(i-GMID) h s} *
e^{tmid s}: the device computes only the iota part (a single [128, G] iota
tile broadcast via 0-stride APs; no per-group broadcast-q DMA at all), and
the host folds e^{tmid s} into vp's V rows and its Z column.  This keeps
the DMA stream at ~20MB/core — the kernel is DMA-stream-bound, so bytes
are the speed currency.

Per group g=(i expert, d dir), all wide single instructions:
  DVE   L[k,(c,b,i)] = iota[i] * hs[k,(c,b)] : one [128, 3072] fp16
        tensor_tensor with 0-stride broadcast APs (hs materialized x4 on
        the host so the last AP dim stays stride-1 and the DVE 2x fp16
        mode applies).
  ACT   EG = exp(L): one wide [128, 3072] activation -> bf16.
  PE    per batch pair: two 6-chunk chains into one 2-bank PSUM tile
        (start=True zeroing is bank-scoped; verified on HW):
        FZ[i_grid, 0:128 | 128] = EG^T @ [fold*V | fold].
  norm  at GRID level, per pair: one reciprocal [G,2] + one tensor_tensor
        with a 0-stride-broadcast reciprocal -> fp16 Fg (g(t_i) is O(1),
        so fp16 is safe — this is why the grid is normalized before
        interpolation).
  interp once per expert: per (b, j): 5-matmul fp16 chain accumulating the
        DIRECTION SUM directly in PSUM: out_j += Wt[d,b,j]^T @ Fg[d,b]
        with fusion weights folded into Wt on the host.  Final [128,128]
        PSUM->SBUF copies (split ACT/DVE) -> fp16 output DMA.  Interp
        chains are dripped between grid chains (pending queue) so phase-2
        work never bunches at expert boundaries.

No per-query softmax normalization anywhere, no collectives.

Sharding: expert-parallel, 2 experts per core (core c owns experts 2c,
2c+1). Outputs land in disjoint slots of the [B, E*P, D] output.
"""

import os
import sys

import numpy as np

sys.path.insert(0, "/opt/trn_rl_repo")

import concourse.bass as bass  # noqa: E402
import concourse.tile as tile  # noqa: E402
from concourse import bacc  # noqa: E402
from concourse import mybir  # noqa: E402
from concourse import bass_utils  # noqa: E402

try:
    from ml_dtypes import bfloat16 as _bf16
except ImportError:  # pragma: no cover
    _bf16 = None

# Problem shape (fixed by the nn.Module).
N_DIR, E, B, P, D, W = 5, 16, 8, 256, 128, 3
EPS = 1e-6
N_CORES = 8
EPC = E // N_CORES          # experts per core = 2
NG = EPC * N_DIR            # groups per core = 10, group g = (i, d)
K = W * P                   # 768 routed keys per query
NCH = K // 128              # 6 k-chunks of 128 partitions
FB = B * P                  # 2048
NT = NCH * B                # 48 V tiles per group
VW = 129                    # V tile width: 128 dcols + ones column
G = 64                      # t-grid points per (d, e, b)
GMID = (G - 1) / 2.0        # iota center
GF = NCH * B * G            # 4608 = logit/exp free size per group
REP = 4                     # host-side s replication factor (96 = 24*4)

F32 = mybir.dt.float32
BF16 = mybir.dt.bfloat16
F16 = mybir.dt.float16

# Exposed for test.py: set True to collect an NTFF profile.
PROFILE = False
LAST_EXEC_NS = None
LAST_TRACE = None

_PROGRAM_CACHE = {}

_AXON_SO = "/opt/axon/libaxon_pjrt.so"


def _ensure_ntff_hook():
    """The container image ships a slim ``antenv`` without ``axon_hooks``;
    register an equivalent module backed by ctypes calls into
    libaxon_pjrt.so so run_bass_kernel_spmd(trace=True) can profile."""
    import sys as _sys
    if "antenv.axon_hooks" in _sys.modules:
        return
    import contextlib
    import ctypes
    import types

    try:
        lib = ctypes.CDLL(_AXON_SO)
    except OSError:
        return
    if not hasattr(lib, "axon_start_nrt_profile"):
        return
    lib.axon_start_nrt_profile.argtypes = [
        ctypes.POINTER(ctypes.c_int64), ctypes.c_size_t]
    lib.axon_start_nrt_profile.restype = ctypes.c_int64
    lib.axon_stop_nrt_profile.argtypes = [ctypes.c_char_p]
    lib.axon_stop_nrt_profile.restype = ctypes.c_int64

    @contextlib.contextmanager
    def _hook(output_dir, device_ids):
        import jax
        jax.devices()
        if device_ids:
            ids = (ctypes.c_int64 * len(device_ids))(*device_ids)
            rc = lib.axon_start_nrt_profile(ids, len(device_ids))
        else:
            rc = lib.axon_start_nrt_profile(None, 0)
        if rc != 0:
            raise RuntimeError(f"axon_start_nrt_profile rc={rc}")
        try:
            yield
        finally:
            n = lib.axon_stop_nrt_profile(str(output_dir).encode())
            print(f"ntff profile: {n} file(s) -> {output_dir}")

    mod = types.ModuleType("antenv.axon_hooks")
    mod.get_axon_ntff_profile_hook = lambda: _hook
    mod.set_axon_ntff_profile_hook = lambda h: None
    _sys.modules["antenv.axon_hooks"] = mod


def build_program(bias_c):
    """Build the SPMD Bass/Tile program (identical on all 8 cores)."""
    from contextlib import ExitStack

    nc = bacc.Bacc("TRN2", target_bir_lowering=False, debug=False,
                   num_devices=N_CORES)

    iot_d = nc.dram_tensor("iot", [128, G], F16, kind="ExternalInput")
    sr_d = nc.dram_tensor("sr", [128, NG * NCH * B * REP], F16,
                          kind="ExternalInput")
    vp_d = nc.dram_tensor("vp", [NG, 128, NT * VW], BF16, kind="ExternalInput")
    wt_d = nc.dram_tensor("wt", [G, NG * FB], F16, kind="ExternalInput")
    out_d = nc.dram_tensor("out", [B, EPC * P, D], F16, kind="ExternalOutput")

    LAG = 2                 # stage B (PE/norm) trails stage A by 2 groups

    with tile.TileContext(nc) as tc, ExitStack() as ctx:
        iot_pool = ctx.enter_context(tc.tile_pool(name="iot", bufs=1))
        sr_pool = ctx.enter_context(tc.tile_pool(name="sr", bufs=1))
        wt_pool = ctx.enter_context(tc.tile_pool(name="wt", bufs=2))
        v_pool = ctx.enter_context(tc.tile_pool(name="vp", bufs=LAG + 3))
        l_pool = ctx.enter_context(tc.tile_pool(name="logit", bufs=2))
        e_pool = ctx.enter_context(tc.tile_pool(name="expt", bufs=LAG + 3))
        fg_pool = ctx.enter_context(tc.tile_pool(name="fg", bufs=40))
        rz_pool = ctx.enter_context(tc.tile_pool(name="rz", bufs=12))
        fo_pool = ctx.enter_context(tc.tile_pool(name="fout", bufs=6))
        gps_pool = ctx.enter_context(
            tc.tile_pool(name="gpsum", bufs=3, space="PSUM"))
        ips_pool = ctx.enter_context(
            tc.tile_pool(name="ipsum", bufs=2, space="PSUM"))

        # iota + ALL the (tiny) hs data land up front so the logit/exp
        # pipeline never queues behind the bulk vp stream.
        iot_sb = iot_pool.tile([128, G], F16)
        nc.sync.dma_start(iot_sb[:, :], iot_d[:, :])
        sr_sb = sr_pool.tile([128, NG * NCH * B * REP], F16)
        nc.sync.dma_start(sr_sb[:, :], sr_d[:, :])

        e_tiles = [None] * NG
        v_tiles = [None] * NG
        wt_tiles = [None] * EPC
        fg_tiles = {}            # (g, b) -> [G, 128] fp16 normalized grid
        pending = []             # (expert, b) interps awaiting emission

        def emit_dma(g):
            """Prefetch group g's bulk V tiles (ahead of use)."""
            v_t = v_pool.tile([128, NT * VW], BF16)
            nc.sync.dma_start(v_t[:, :], vp_d[g, :, :])
            v_tiles[g] = v_t
            if g % N_DIR == 3:
                # this expert's interp weights; first used by phase 2
                i = g // N_DIR
                wt_t = wt_pool.tile([G, N_DIR * FB], F16)
                nc.sync.dma_start(
                    wt_t[:, :],
                    wt_d[:, i * N_DIR * FB:(i + 1) * N_DIR * FB])
                wt_tiles[i] = wt_t

        def emit_interp(i, b):
            """Quintic interp + direction sum in one PSUM chain, then
            PSUM->SBUF copy and the output DMA, for (expert i, batch b)."""
            wt_sb = wt_tiles[i]
            for j in range(2):
                ps2 = ips_pool.tile([128, 128], F32)
                for d in range(N_DIR):
                    gg = i * N_DIR + d
                    wt_ap = wt_sb[:, (d * B + b) * P + j * 128:
                                  (d * B + b) * P + j * 128 + 128]
                    nc.tensor.matmul(
                        ps2[:, :],
                        wt_ap,
                        fg_tiles[(gg, b)][:, :],
                        start=(d == 0), stop=(d == N_DIR - 1),
                    )
                fo = fo_pool.tile([128, 128], F16)
                if j == 0:
                    nc.scalar.activation(
                        fo[:, :], ps2[:, :],
                        mybir.ActivationFunctionType.Copy,
                        bias=0.0, scale=1.0)
                else:
                    nc.vector.tensor_scalar(
                        fo[:, :], ps2[:, :], 1.0, None,
                        mybir.AluOpType.mult)
                nc.sync.dma_start(
                    out_d[b, i * P + j * 128:i * P + j * 128 + 128, :],
                    fo[:, :])

        emit_dma(0)
        for g in range(NG + LAG):
            if g + 1 < NG:
                emit_dma(g + 1)
            if g < NG:
                # ---- stage A: logits + exp for group g ----
                l_t = l_pool.tile([128, GF], F16)
                # l[k,(c,b,i)] = (i - GMID) * (h_b * s_kcb); the remaining
                # e^{tmid*s} factor of e^{t_i s} is folded into vp's rows
                # (and its Z column) on the host.
                iot_ap = iot_sb[:, :].unsqueeze(1).broadcast_to(
                    [128, NCH * B, G])
                base = g * NCH * B * REP
                sr_ap = sr_sb[:, base:base + NCH * B * REP]
                sr_ap = sr_ap.rearrange("p (cb r) -> p cb r", r=REP)
                sr_ap = sr_ap.unsqueeze(2).broadcast_to(
                    [128, NCH * B, G // REP, REP])
                nc.vector.tensor_tensor(
                    l_t[:, :].rearrange("p (cb i) -> p cb i", i=G),
                    iot_ap, sr_ap, mybir.AluOpType.mult)

                e_t = e_pool.tile([128, GF], BF16)
                nc.scalar.activation(
                    e_t[:, :], l_t[:, :],
                    mybir.ActivationFunctionType.Exp,
                    bias=float(bias_c), scale=1.0,
                )
                e_tiles[g] = e_t

            if g >= LAG:
                # ---- stage B: grid chains + grid-normalize for g-LAG ----
                # Two chains share one 2-bank PSUM tile (chain u=0 in bank 0,
                # u=1 in bank 1; start=True zeroing is bank-scoped), so the
                # recip and the normalize-copy each cover a PAIR of batches.
                gp = g - LAG
                e_t = e_tiles[gp]
                v_t = v_tiles[gp]
                i, d = gp // N_DIR, gp % N_DIR
                for bp in range(0, B, 2):
                    ps = gps_pool.tile([128, 1024], F32)
                    for u in range(2):
                        b = bp + u
                        for c in range(NCH):
                            nc.tensor.matmul(
                                ps[0:G, u * 512:u * 512 + VW],
                                e_t[:, (c * B + b) * G:(c * B + b + 1) * G],
                                v_t[:, (c * B + b) * VW:
                                    (c * B + b + 1) * VW],
                                start=(c == 0), stop=(c == NCH - 1),
                            )
                    psv = ps[0:G, :].rearrange("p (u v) -> p u v", v=512)
                    rz2 = rz_pool.tile([128, 2], F32)
                    nc.vector.reciprocal(rz2[0:G, :], psv[:, :, 128:129])
                    fg2 = fg_pool.tile([G, 2 * 128], F16)
                    # normalized grid rows: g(t_i) = F/Z, O(1) -> fp16 safe.
                    rz_ap = rz2[0:G, :].unsqueeze(2).broadcast_to(
                        [G, 2, 128])
                    nc.vector.tensor_tensor(
                        fg2[:, :].rearrange("p (u v) -> p u v", v=128),
                        psv[:, :, 0:128], rz_ap, mybir.AluOpType.mult)
                    fg_tiles[(gp, bp)] = fg2[:, 0:128]
                    fg_tiles[(gp, bp + 1)] = fg2[:, 128:256]
                    # Drip pending interp chains between grid chains so the
                    # phase-2 work never bunches at expert boundaries.
                    if gp == NG - 1:
                        pending.append((i, bp))
                        pending.append((i, bp + 1))
                        if bp >= 2:
                            emit_interp(*pending.pop(0))
                            emit_interp(*pending.pop(0))
                    elif pending:
                        emit_interp(*pending.pop(0))
                if d == N_DIR - 1 and gp < NG - 1:
                    pending.extend((i, b) for b in range(B))
                if gp == NG - 1:
                    while pending:
                        emit_interp(*pending.pop(0))

    nc.compile()
    return nc


def host_prep(Q_aff, K_aff, V, betas, temperature, fusion_w, routes):
    """Shard + gather + layout inputs for the 8 cores. Returns
    (in_maps, bias_c)."""
    Q_aff = np.asarray(Q_aff, np.float32)
    K_aff = np.asarray(K_aff, np.float32)
    V = np.asarray(V, np.float32)
    betas = np.asarray(betas, np.float32)
    temperature = np.asarray(temperature, np.float32)
    fusion_w = np.asarray(fusion_w, np.float32)
    routes = np.asarray(routes)

    T = abs(float(temperature[0])) + EPS
    fw = np.exp(fusion_w - fusion_w.max())
    fw = (fw / fw.sum()).astype(np.float32)          # softmax(fusion_w)

    ar = np.arange(E)
    is_self = routes == ar[:, None]
    gates = 1.0 / (1.0 + np.exp(-betas[ar[:, None], routes]))
    beta = np.where(is_self, 1.0, gates).astype(np.float32)   # [E, W]

    # S[d, e, b, k] with k = w*P + p'
    nbK = K_aff[:, routes]                            # [d, E, W, b, P]
    S = nbK * beta[None, :, :, None, None] / np.float32(T)
    S = np.moveaxis(S, 2, 3).reshape(N_DIR, E, B, K)  # [d, E, b, K]

    # t-grids per (d, e, b): G points spanning [qmin, qmax] with 2.5-tap
    # margin so every q_p sits in the interior of a 6-tap stencil.
    qmin = Q_aff.min(axis=3)                          # [d, E, B]
    qmax = Q_aff.max(axis=3)
    h = np.maximum((qmax - qmin) / (G - 6), 1e-5)
    tgrid = (qmin[..., None] + (np.arange(G, dtype=np.float32) - 2.5)
             * h[..., None]).astype(np.float32)       # [d, E, B, G]
    # Factorization e^{t_i s} = e^{(i-47.5) h s} * e^{tmid s} with
    # tmid = t0 + 45h: the device computes only the iota part; the
    # e^{tmid s} factor is folded into vp's V rows and its Z column.
    tmid = (qmin + (GMID - 2.5) * h).astype(np.float32)       # [d, E, B]

    # Max |grid logit| of the iota part: decide the exp shift (bf16 range
    # guard; e^88 overflows bf16).
    sabs = np.abs(S).max(axis=3)
    maxarg = float((GMID * h * sabs).max())
    bias_c = 0.0 if maxarg < 80.0 else -(maxarg - 60.0)

    # Quintic Lagrange interp weights W[p, G] per (d, e, b), scaled by the
    # fusion weight so the direction sum happens inside PSUM chains.
    cell = ((Q_aff - tgrid[..., 0:1]) / h[..., None]).astype(np.int64)
    cell = np.clip(cell, 2, G - 4)                    # [d, E, B, P]
    i0 = cell - 2
    taps = i0[..., None] + np.arange(6)               # [d, E, B, P, 6]
    xs = np.take_along_axis(
        tgrid[..., None, :], taps, axis=4)            # [d, E, B, P, 6]
    q = Q_aff[..., None]                              # [d, E, B, P, 1]
    wq = np.ones((N_DIR, E, B, P, 6), np.float64)
    for a in range(6):
        for c in range(6):
            if c == a:
                continue
            wq[..., a] *= (q[..., 0] - xs[..., c]) / (xs[..., a] - xs[..., c])
    Wfull = np.zeros((N_DIR, E, B, P, G), np.float32)
    np.put_along_axis(Wfull, taps, wq.astype(np.float32), axis=4)
    Wfull *= fw[:, None, None, None, None]

    if _bf16 is None:
        raise RuntimeError("ml_dtypes.bfloat16 required")

    iot = np.broadcast_to(
        (np.arange(G, dtype=np.float32) - GMID).astype(np.float16),
        (128, G)).copy()

    in_maps = []
    for core in range(N_CORES):
        experts = [EPC * core + i for i in range(EPC)]

        sr = np.empty((128, NG * NCH * B * REP), np.float16)
        vp = np.empty((NG, 128, NT, VW), np.float32)
        wt = np.empty((G, NG * FB), np.float16)
        for i, e in enumerate(experts):
            for d in range(N_DIR):
                g = i * N_DIR + d
                # sr holds h_b * s so the iota multiply lands at (i-47.5)*h*s
                hs_mat = (S[d, e] * h[d, e][:, None]).reshape(
                    B, NCH, 128).transpose(2, 1, 0)
                sr[:, g * NCH * B * REP:(g + 1) * NCH * B * REP] = np.repeat(
                    hs_mat.reshape(128, NCH * B).astype(np.float16),
                    REP, axis=1)
                # wt[i_grid, g*FB + b*P + p] = fw[d] * W[d,e,b,p,i_grid]
                wt[:, g * FB:(g + 1) * FB] = (
                    Wfull[d, e].reshape(FB, G).T.astype(np.float16))
                for c in range(NCH):
                    w, half = c // 2, c % 2
                    f = int(routes[e, w])
                    # fold e^{tmid s} into the V rows and the Z column
                    fold = np.exp(
                        tmid[d, e][:, None]
                        * S[d, e, :, c * 128:(c + 1) * 128]
                    ).astype(np.float32)              # [B, 128]
                    vp[g, :, c * B:(c + 1) * B, :D] = (
                        fold[:, :, None]
                        * V[d, f, :, half * 128:(half + 1) * 128, :]
                    ).transpose(1, 0, 2)
                    vp[g, :, c * B:(c + 1) * B, D] = fold.T
        in_maps.append({
            "iot": iot,
            "sr": sr,
            "vp": vp.reshape(NG, 128, NT * VW).astype(_bf16),
            "wt": wt,
        })
    return in_maps, bias_c


def kernel(**inputs):
    global LAST_EXEC_NS, LAST_TRACE
    in_maps, bias_c = host_prep(**inputs)

    key = (bias_c,)
    nc = _PROGRAM_CACHE.get(key)
    if nc is None:
        nc = build_program(bias_c)
        _PROGRAM_CACHE[key] = nc

    if PROFILE:
        _ensure_ntff_hook()
    res = bass_utils.run_bass_kernel_spmd(
        nc, in_maps, list(range(N_CORES)), trace=PROFILE)
    LAST_EXEC_NS = res.exec_time_ns
    LAST_TRACE = getattr(res, "instructions_and_trace", None)

    out = np.empty((B, E * P, D), np.float32)
    for core in range(N_CORES):
        out[:, EPC * core * P:(EPC * core + EPC) * P, :] = (
            res.results[core]["out"].astype(np.float32))
    return out
